# revision 7
# baseline (speedup 1.0000x reference)
"""Trainium2 Bass kernel for nn_CrossAttention_72121090834620.

Math (B=2, N=160, D=4096, H=4, DH=1024, S=DH**-0.5):
    q = y @ Wq.T                          (B, N, D)
    k = (x @ Wk.T) -> heads               (B, H, N, DH)
    kkt[b,h,i,j] = sum_d k[b,h,i,d]k[b,h,j,d] * S
    dots[b,h,i,j] = sum_m q[b,m,i] kkt[b,h,j,m] * S
    out = softplus(dots).reshape(40, 32, 4096)

Sharding (8 cores, tensor parallel over the projection output dim):
    core c owns output dims [c*512, (c+1)*512) of both projections.
    - kT projection: kT[d_l, nb] with d on partitions (512 local dims)
    - partial kkt for its half-head (core c covers half of head c//2)
    - AllGather of the 8 partial kkts (205KB/rank), pair-summed locally
    - q projection into per-batch row tiles q[b][m, i_l]
    - dots computed transposed (dotsT[j, i_l]) using kkt symmetry so the
      moving operand is 512 wide; softplus = Ln(Exp(S^2 x)+1) on ScalarE
    - output per core: (2, 4, 160, 512) = dots[b,h,:,i_l] transposed

Host pre/post: transposes + shard slicing + final gather/reshape (numpy).
"""

import numpy as np

import concourse.mybir as mybir
import concourse.tile as tile
from concourse import bacc, bass_utils

N_CORES = 8
B, N, D, H = 2, 160, 4096, 4
DH = D // H
S2 = 1.0 / float(DH)  # SCALE applied twice
NB = B * N  # 320
DSH = D // N_CORES  # 512 output dims per core
P = 128
MK = D // P  # 32 contraction chunks of 128
G = 4  # contraction chunks per streamed DMA
FP32 = mybir.dt.float32

# row splits of the 160-long m/j axis (all tiles base-0: the PE requires
# lhsT/rhs to share base_partition)
J_CHUNKS = [(128, 0), (32, 128)]


def _emit_body(nc, pools, aps, r):
    """Emit one full compute pass (rep r gets unique tile names)."""
    stream, keep, work, psum, dram = pools
    xT_r, yT_r, wkT_r, wqT_r, out = aps

    # ---------------- kT projection: kT[d_l, nb] ----------------
    kT_sb = keep.tile([P, 4, NB], FP32, name=f"kT_sb_{r}")
    psum_kt = [
        psum.tile([P, NB], FP32, tag="acc", bufs=4, name=f"pkt{dt}_{r}")
        for dt in range(4)
    ]
    for g in range(MK // G):
        xt_g = stream.tile([P, G, NB], FP32, tag="xs", name=f"xt{g}_{r}")
        nc.sync.dma_start(xt_g[:], xT_r[:, g * G : (g + 1) * G, :])
        wk_g = stream.tile([P, G, DSH], FP32, tag="ws", name=f"wk{g}_{r}")
        nc.sync.dma_start(wk_g[:], wkT_r[:, g * G : (g + 1) * G, :])
        for s in range(G):
            mc = g * G + s
            for dt in range(4):
                nc.tensor.matmul(
                    psum_kt[dt][:],
                    lhsT=wk_g[:, s, dt * P : (dt + 1) * P],
                    rhs=xt_g[:, s, :],
                    start=(mc == 0),
                    stop=(mc == MK - 1),
                )
    for dt in range(4):
        nc.vector.tensor_copy(kT_sb[:, dt, :], psum_kt[dt][:])

    # ------------- partial kkt for this core's 512 dims -------------
    bounce_in = dram.tile([NB, N], FP32, name=f"bounce_in_{r}")
    for b in range(B):
        for isz, i0 in J_CHUNKS:
            pk = psum.tile([isz, N], FP32, tag="acc", bufs=4, name=f"pkk{b}_{i0}_{r}")
            for dc in range(4):
                nc.tensor.matmul(
                    pk[:],
                    lhsT=kT_sb[:, dc, b * N + i0 : b * N + i0 + isz],
                    rhs=kT_sb[:, dc, b * N : (b + 1) * N],
                    start=(dc == 0),
                    stop=(dc == 3),
                )
            kk_sb = work.tile([isz, N], FP32, tag="kkc", name=f"kkc{b}_{i0}_{r}")
            nc.vector.tensor_copy(kk_sb[:], pk[:])
            nc.sync.dma_start(bounce_in[b * N + i0 : b * N + i0 + isz, :], kk_sb[:])

    # ---------------- AllGather the 8 partial kkts ----------------
    ag_out = dram.tile(
        [N_CORES * NB, N], FP32, addr_space="Shared", name=f"ag_out_{r}"
    )
    nc.gpsimd.collective_compute(
        "AllGather",
        mybir.AluOpType.bypass,
        replica_groups=[list(range(N_CORES))],
        ins=[bounce_in.opt()],
        outs=[ag_out.opt()],
    )

    # ------------- q projection (overlaps the AllGather) -------------
    q_sb = {
        (b, ci): keep.tile([rsz, DSH], FP32, name=f"q_sb{b}_{ci}_{r}")
        for b in range(B)
        for ci, (rsz, _) in enumerate(J_CHUNKS)
    }
    psum_q = {
        (b, ci): psum.tile([rsz, DSH], FP32, tag="big", bufs=4, name=f"pq{b}_{ci}_{r}")
        for b in range(B)
        for ci, (rsz, _) in enumerate(J_CHUNKS)
    }
    for g in range(MK // G):
        yt_g = stream.tile([P, G, NB], FP32, tag="xs", name=f"yt{g}_{r}")
        nc.sync.dma_start(yt_g[:], yT_r[:, g * G : (g + 1) * G, :])
        wq_g = stream.tile([P, G, DSH], FP32, tag="ws", name=f"wq{g}_{r}")
        nc.sync.dma_start(wq_g[:], wqT_r[:, g * G : (g + 1) * G, :])
        for s in range(G):
            mc = g * G + s
            for b in range(B):
                for ci, (rsz, r0) in enumerate(J_CHUNKS):
                    nc.tensor.matmul(
                        psum_q[b, ci][:],
                        lhsT=yt_g[:, s, b * N + r0 : b * N + r0 + rsz],
                        rhs=wq_g[:, s, :],
                        start=(mc == 0),
                        stop=(mc == MK - 1),
                    )
    for key, tile_ in q_sb.items():
        nc.vector.tensor_copy(tile_[:], psum_q[key][:])

    # ---- AG readback + pair sum: kkt[b,h] = part[2h] + part[2h+1] ----
    kkt_tiles = {}
    for b in range(B):
        for h in range(H):
            for ci, (rsz, r0) in enumerate(J_CHUNKS):
                base0 = 2 * h * NB + b * N + r0
                base1 = (2 * h + 1) * NB + b * N + r0
                tA = work.tile([rsz, N], FP32, tag="aga", name=f"ta{b}{h}{ci}_{r}")
                nc.sync.dma_start(tA[:], ag_out[base0 : base0 + rsz, :])
                tB = work.tile([rsz, N], FP32, tag="agb", name=f"tb{b}{h}{ci}_{r}")
                nc.sync.dma_start(tB[:], ag_out[base1 : base1 + rsz, :])
                kt = keep.tile([rsz, N], FP32, name=f"kkt{b}{h}{ci}_{r}")
                nc.vector.tensor_add(kt[:], tA[:], tB[:])
                kkt_tiles[b, h, ci] = kt

    # ---------------- dots (transposed) + softplus + out ----------------
    # dotsT[j, i_l] = sum_m kkt[b,h][m, j] * q[b][m, i_l]  (kkt symmetric)
    q_rhs = {b: [q_sb[b, 0][:], q_sb[b, 1][:]] for b in range(B)}
    for b in range(B):
        for h in range(H):
            for jsz, j0 in J_CHUNKS:
                pd = psum.tile(
                    [jsz, DSH], FP32, tag="big", bufs=4, name=f"pd{b}{h}{j0}_{r}"
                )
                for ci in range(2):
                    nc.tensor.matmul(
                        pd[:],
                        lhsT=kkt_tiles[b, h, ci][:, j0 : j0 + jsz],
                        rhs=q_rhs[b][ci],
                        start=(ci == 0),
                        stop=(ci == 1),
                    )
                # softplus(S2*x) = ln(1 + exp(S2*x)); Softplus itself is not
                # in the gen3 act tables but Exp+Ln share one table.
                ex = work.tile([jsz, DSH], FP32, tag="ex", name=f"ex{b}{h}{j0}_{r}")
                nc.scalar.activation(
                    ex[:], pd[:], mybir.ActivationFunctionType.Exp, scale=S2
                )
                ot = work.tile([jsz, DSH], FP32, tag="ot", name=f"ot{b}{h}{j0}_{r}")
                nc.scalar.activation(
                    ot[:], ex[:], mybir.ActivationFunctionType.Ln, bias=1.0
                )
                nc.sync.dma_start(out.ap()[b, h, j0 : j0 + jsz, :], ot[:])


def build_nc(reps=1):
    nc = bacc.Bacc(num_devices=N_CORES, name="xattn", debug=False)
    xT = nc.dram_tensor("xT", [D, NB], FP32, kind="ExternalInput")
    yT = nc.dram_tensor("yT", [D, NB], FP32, kind="ExternalInput")
    wkT = nc.dram_tensor("wkT", [D, DSH], FP32, kind="ExternalInput")
    wqT = nc.dram_tensor("wqT", [D, DSH], FP32, kind="ExternalInput")
    out = nc.dram_tensor("out", [B, H, N, DSH], FP32, kind="ExternalOutput")

    xT_r = xT.ap().rearrange("(o p) f -> p o f", p=P)  # [128, 32, 320]
    yT_r = yT.ap().rearrange("(o p) f -> p o f", p=P)
    wkT_r = wkT.ap().rearrange("(o p) f -> p o f", p=P)  # [128, 32, 512]
    wqT_r = wqT.ap().rearrange("(o p) f -> p o f", p=P)

    with tile.TileContext(nc) as tc:
        with (
            tc.tile_pool(name="stream", bufs=3) as stream,
            tc.tile_pool(name="keep", bufs=1) as keep,
            tc.tile_pool(name="work", bufs=4) as work,
            tc.tile_pool(name="psum", bufs=1, space="PSUM") as psum,
            tc.tile_pool(name="dram", bufs=1, space="DRAM") as dram,
        ):
            pools = (stream, keep, work, psum, dram)
            aps = (xT_r, yT_r, wkT_r, wqT_r, out)
            for r in range(reps):
                _emit_body(nc, pools, aps, r)

    nc.compile()
    return nc


def prep_in_maps(x, y, Wq, Wk):
    x = np.asarray(x, dtype=np.float32)
    y = np.asarray(y, dtype=np.float32)
    Wq = np.asarray(Wq, dtype=np.float32)
    Wk = np.asarray(Wk, dtype=np.float32)
    xT = np.ascontiguousarray(x.reshape(NB, D).T)
    yT = np.ascontiguousarray(y.reshape(NB, D).T)
    WqT = np.ascontiguousarray(Wq.T)
    WkT = np.ascontiguousarray(Wk.T)
    in_maps = []
    for c in range(N_CORES):
        sl = slice(c * DSH, (c + 1) * DSH)
        in_maps.append(
            {
                "xT": xT,
                "yT": yT,
                "wqT": np.ascontiguousarray(WqT[:, sl]),
                "wkT": np.ascontiguousarray(WkT[:, sl]),
            }
        )
    return in_maps


def assemble(results):
    full = np.empty((B, H, D, N), dtype=np.float32)
    for c in range(N_CORES):
        shard = results[c]["out"]  # (B, H, N, DSH) = dots[b,h,j,i_l]
        full[:, :, c * DSH : (c + 1) * DSH, :] = shard.transpose(0, 1, 3, 2)
    return full.reshape(40, 32, D)


def kernel(x, y, Wq, Wk):
    nc = build_nc()
    in_maps = prep_in_maps(x, y, Wq, Wk)
    res = bass_utils.run_bass_kernel_spmd(nc, in_maps, core_ids=list(range(N_CORES)))
    return assemble(res.results)
